# revision 2
# baseline (speedup 1.0000x reference)
"""Trainium2 Bass kernel for nn_CAWeightedFusion.

Math note: in the reference, ra/ca are softmaxed over the flattened spatial
axis N=H*W and then immediately mean-pooled over that same axis. A softmax
row sums to exactly 1, so mean(ra) = mean(ca) = 1/N elementwise and the whole
QKV/attention pipeline cancels out of the output:

    g[b,c] = mean_hw(rgb[b,c]) + mean_hw(chm[b,c]) + 2/N
    out    = sigmoid(relu(g @ w_mlp1.T) @ w_mlp2.T)[:, :, None, None]

What remains is a memory-bound spatial reduction plus a tiny MLP. We go
batch-parallel: core b reduces batch b (rgb+chm, shipped bf16), fusing the
first MLP layer into the reduction as 64 PSUM-accumulated matmuls
(w1_chunk.T[128,24] @ x_chunk[128,512]), then one free-axis reduce, a
bias+relu (the 1/N scale and the 2/N constant folded into scale/bias), the
1x24 second layer, and a sigmoid.
"""

import numpy as np
import ml_dtypes

B, C, HW = 8, 512, 4096
NCORES = 8
HID = 24

_CACHE = {}


def _build_program():
    import concourse.bacc as bacc
    import concourse.bass as bass
    import concourse.mybir as mybir
    import concourse.tile as tile

    bf16 = mybir.dt.bfloat16
    f32 = mybir.dt.float32
    ts = bass.ts

    nc = bacc.Bacc(
        "TRN2",
        target_bir_lowering=False,
        debug=False,
        enable_asserts=False,
        num_devices=NCORES,
    )

    xr = nc.dram_tensor("xr", [C, HW], bf16, kind="ExternalInput")
    xc = nc.dram_tensor("xc", [C, HW], bf16, kind="ExternalInput")
    # wt[:, 24k:24k+24] = w_mlp1[:, 128k:128k+128].T  (k = 0..3)
    wt = nc.dram_tensor("wt", [128, 4 * HID], bf16, kind="ExternalInput")
    b1 = nc.dram_tensor("b1", [HID, 1], f32, kind="ExternalInput")
    w2t = nc.dram_tensor("w2t", [HID, 1], f32, kind="ExternalInput")
    out = nc.dram_tensor("out", [1, 1], f32, kind="ExternalOutput")

    with tile.TileContext(nc) as tc:
        with (
            tc.tile_pool(name="xp", bufs=3) as xp,
            tc.tile_pool(name="cst", bufs=1) as cst,
            tc.tile_pool(name="acc", bufs=1, space="PSUM") as accp,
            tc.tile_pool(name="eps", bufs=1, space="PSUM") as epsp,
            tc.tile_pool(name="sb", bufs=1) as sb,
        ):
            wt_t = cst.tile([128, 4 * HID], bf16)
            nc.sync.dma_start(wt_t[:], wt[:])
            b1_t = cst.tile([HID, 1], f32)
            nc.sync.dma_start(b1_t[:], b1[:])
            w2_t = cst.tile([HID, 1], f32)
            nc.sync.dma_start(w2_t[:], w2t[:])

            acc = accp.tile([HID, 512], f32)
            for t in range(8):
                src = xr if t < 4 else xc
                rs = t % 4
                xt = xp.tile([128, HW], bf16)
                nc.sync.dma_start(xt[:], src[ts(rs, 128), :])
                for j in range(8):
                    nc.tensor.matmul(
                        acc[:],
                        wt_t[:, ts(rs, HID)],
                        xt[:, ts(j, 512)],
                        start=(t == 0 and j == 0),
                        stop=(t == 7 and j == 7),
                    )

            s = sb.tile([HID, 1], f32)
            nc.vector.reduce_sum(s[:], acc[:], axis=mybir.AxisListType.X)
            h1 = sb.tile([HID, 1], f32)
            nc.scalar.activation(
                h1[:], s[:], mybir.ActivationFunctionType.Relu,
                bias=b1_t[:], scale=1.0 / HW,
            )
            g2 = epsp.tile([1, 1], f32)
            nc.tensor.matmul(g2[:], h1[:], w2_t[:], start=True, stop=True)
            gate = sb.tile([1, 1], f32)
            nc.scalar.activation(gate[:], g2[:], mybir.ActivationFunctionType.Sigmoid)
            nc.sync.dma_start(out[:], gate[:])

    nc.compile()
    return nc


def kernel(rgb, chm, w_rgb_qkv, b_rgb_qkv, w_chm_qkv, b_chm_qkv, w_mlp1, w_mlp2):
    from concourse.bass_utils import run_bass_kernel_spmd

    if "nc" not in _CACHE:
        _CACHE["nc"] = _build_program()
    nc = _CACHE["nc"]

    bf16 = ml_dtypes.bfloat16
    w1 = np.asarray(w_mlp1, dtype=np.float32)          # [24, 512]
    wt = np.empty((128, 4 * HID), dtype=bf16)
    for k in range(4):
        wt[:, k * HID:(k + 1) * HID] = w1[:, k * 128:(k + 1) * 128].T.astype(bf16)
    b1 = (2.0 / HW) * w1.sum(axis=1, dtype=np.float64)
    b1 = b1.astype(np.float32).reshape(HID, 1)
    w2t = np.asarray(w_mlp2, dtype=np.float32).reshape(HID, 1)

    rgb = np.asarray(rgb).reshape(B, C, HW)
    chm = np.asarray(chm).reshape(B, C, HW)
    in_maps = []
    for b in range(B):
        in_maps.append({
            "xr": rgb[b].astype(bf16),
            "xc": chm[b].astype(bf16),
            "wt": wt,
            "b1": b1,
            "w2t": w2t,
        })

    res = run_bass_kernel_spmd(nc, in_maps, core_ids=list(range(NCORES)))
    _CACHE["last_results"] = res

    gates = np.stack([res.results[b]["out"].reshape(()) for b in range(B)])
    return gates.reshape(B, 1, 1, 1).astype(np.float32)
